# revision 16
# baseline (speedup 1.0000x reference)
"""Trainium2 Bass kernel for hetero GNN (2x SAGEConv layers + in/out proj).

Full inputs in, full output out. Internally: dst-node sharding across 8
NeuronCores, edge bucketing by (dst block of 128, src quadrant) on host,
device-side gather via SWDGE dma_gather, segment-mean via one-hot matmul
accumulated in PSUM, AllGather collectives for the shared node tables.
"""

import math

import numpy as np

import concourse.bacc as bacc
import concourse.bass as bass
import concourse.mybir as mybir
from concourse import tile
from concourse.bass_utils import run_bass_kernel_spmd

FP32 = mybir.dt.float32
BF16 = mybir.dt.bfloat16
I16 = mybir.dt.int16
AF = mybir.ActivationFunctionType
ALU = mybir.AluOpType

BF16_NP = mybir.dt.np(BF16)


def full_cfg():
    return dict(
        N=100000,
        E=1600000,
        DA=300,
        DU=64,
        H=64,
        OUT=2,
        n_cores=8,
        shard=12544,  # 98 * 128 per-core dst shard
        cq_min=5,
    )


# ----------------------------------------------------------------------------
# Host-side edge preprocessing
# ----------------------------------------------------------------------------


def prep_edges(src, dst, cfg):
    """Bucket edges by (dst block of 128, src quadrant); build gather index /
    one-hot slot / reciprocal-degree arrays per core.

    Returns (CQ, per_core list of dicts with idx_w/slot_w/rval_w).
    """
    N, shard, n_cores = cfg["N"], cfg["shard"], cfg["n_cores"]
    NPAD = n_cores * shard
    QN = NPAD // 4
    assert QN < 32768, QN
    NBLK = shard // 128

    src = np.asarray(src, dtype=np.int64)
    dst = np.asarray(dst, dtype=np.int64)
    deg = np.bincount(dst, minlength=N).astype(np.float64)
    recip = (1.0 / np.maximum(deg, 1.0)).astype(np.float32)

    blk = dst >> 7  # global 128-block id
    quad = src // QN
    n_cells = n_cores * NBLK * 4
    cell = blk * 4 + quad
    # sort edges by cell (order within a cell is irrelevant)
    order = np.argsort(cell, kind="stable")
    c_src = src[order]
    c_dst = dst[order]
    c_cell = cell[order]
    starts = np.searchsorted(c_cell, np.arange(n_cells))
    cnts = np.bincount(c_cell, minlength=n_cells)
    CQ = max(cfg["cq_min"], int(math.ceil(cnts.max() / 128)))
    CB = 4 * CQ

    j = np.arange(len(c_src)) - starts[c_cell]  # position within cell
    loc_idx = (c_src - quad[order] * QN).astype(np.int16)
    slot_val = (c_dst & 127).astype(np.float32)
    rval_val = recip[c_dst]

    b_local_all = (c_cell // 4) % NBLK
    q_all = c_cell % 4
    core_all = c_cell // (4 * NBLK)

    per_core = []
    for c in range(n_cores):
        m = core_all == c
        bl = b_local_all[m]
        q = q_all[m]
        jj = j[m]
        # gather idx array, 16-partition wrapped, replicated 8x
        idx_w = np.zeros((128, NBLK * 4 * CQ * 8), dtype=np.int16)
        col = (bl * 4 + q) * (CQ * 8) + jj // 16
        row = jj % 16
        for g in range(8):
            idx_w[row + 16 * g, col] = loc_idx[m]
        # slot / recip-val arrays: [128, NBLK*CB]
        slot_w = np.full((128, NBLK * CB), 999.0, dtype=np.float32)
        rval_w = np.zeros((128, NBLK * CB), dtype=np.float32)
        colS = bl * CB + q * CQ + jj // 128
        rowS = jj % 128
        slot_w[rowS, colS] = slot_val[m]
        rval_w[rowS, colS] = rval_val[m]
        per_core.append(dict(idx_w=idx_w, slot_w=slot_w, rval_w=rval_w))
    return CQ, per_core


def _lin_bf16(w):
    """[out,in] fp32 -> lhsT layout [in,out] bf16."""
    return np.ascontiguousarray(w.T).astype(BF16_NP)


def _bias_col(b):
    return np.asarray(b, np.float32).reshape(-1, 1)


# ----------------------------------------------------------------------------
# Device program
# ----------------------------------------------------------------------------


def build_program(cfg, CQp, CQb, reps=1):
    N, DA, DU, H, OUT = cfg["N"], cfg["DA"], cfg["DU"], cfg["H"], cfg["OUT"]
    n_cores, shard = cfg["n_cores"], cfg["shard"]
    NPAD = n_cores * shard
    QN = NPAD // 4
    NBLK = shard // 128
    CBp, CBb = 4 * CQp, 4 * CQb
    DA_PAD = ((DA + 15) // 16) * 16  # 304
    KA = [(k, min(128, DA_PAD - k)) for k in range(0, DA_PAD, 128)]
    TW = 512  # in-proj / head tile width
    n_tw = [(t, min(TW, shard - t)) for t in range(0, shard, TW)]

    nc = bacc.Bacc("TRN2", debug=False)

    # ---- I/O ----
    xaT = nc.dram_tensor("xaT", [DA_PAD, shard], BF16, kind="ExternalInput")
    xuT = nc.dram_tensor("xuT", [DU, shard], BF16, kind="ExternalInput")
    w_in_aT = nc.dram_tensor("w_in_aT", [DA_PAD, H], BF16, kind="ExternalInput")
    b_in_a = nc.dram_tensor("b_in_a", [H, 1], FP32, kind="ExternalInput")
    w_in_uT = nc.dram_tensor("w_in_uT", [DU, H], BF16, kind="ExternalInput")
    b_in_u = nc.dram_tensor("b_in_u", [H, 1], FP32, kind="ExternalInput")
    convw = {}
    for et in ("c1p", "c1b", "c2p"):
        convw[et] = (
            nc.dram_tensor(f"{et}_wlT", [H, H], BF16, kind="ExternalInput"),
            nc.dram_tensor(f"{et}_bl", [H, 1], FP32, kind="ExternalInput"),
            nc.dram_tensor(f"{et}_wrT", [H, H], BF16, kind="ExternalInput"),
        )
    w_outT = nc.dram_tensor("w_outT", [H, OUT], BF16, kind="ExternalInput")
    b_out = nc.dram_tensor("b_out", [OUT, 1], FP32, kind="ExternalInput")
    iota_in = nc.dram_tensor("iota", [128, 128], FP32, kind="ExternalInput")
    ident_in = nc.dram_tensor("ident", [128, 128], BF16, kind="ExternalInput")
    idx_p = nc.dram_tensor("idx_p", [128, NBLK * 4 * CQp * 8], I16, kind="ExternalInput")
    slot_p = nc.dram_tensor("slot_p", [128, NBLK * CBp], FP32, kind="ExternalInput")
    rval_p = nc.dram_tensor("rval_p", [128, NBLK * CBp], FP32, kind="ExternalInput")
    idx_b = nc.dram_tensor("idx_b", [128, NBLK * 4 * CQb * 8], I16, kind="ExternalInput")
    slot_b = nc.dram_tensor("slot_b", [128, NBLK * CBb], FP32, kind="ExternalInput")
    rval_b = nc.dram_tensor("rval_b", [128, NBLK * CBb], FP32, kind="ExternalInput")
    out_d = nc.dram_tensor("out", [OUT, shard], FP32, kind="ExternalOutput")

    # internal HBM
    u_shard = nc.dram_tensor("u_shard", [shard, 128], BF16)
    a_shard = nc.dram_tensor("a_shard", [shard, 128], BF16)
    u1_shard = nc.dram_tensor("u1_shard", [shard, 128], BF16)
    u_rm = nc.dram_tensor("u_rm", [NPAD, 128], BF16, addr_space="Shared")
    a_rm = nc.dram_tensor("a_rm", [NPAD, 128], BF16, addr_space="Shared")
    u1_rm = nc.dram_tensor("u1_rm", [NPAD, 128], BF16, addr_space="Shared")
    groups = [list(range(n_cores))]

    from contextlib import ExitStack

    with tile.TileContext(nc) as tc, ExitStack() as _stack:
        cpool = _stack.enter_context(tc.tile_pool(name="const", bufs=1))
        # resident constants
        iota_sb = cpool.tile([128, 128], FP32, tag="iota")
        ident_sb = cpool.tile([128, 128], BF16, tag="ident")
        nc.sync.dma_start(iota_sb[:], iota_in[:])
        nc.sync.dma_start(ident_sb[:], ident_in[:])

        def load_const(t, shape, dtype, tag):
            s = cpool.tile(shape, dtype, tag=tag)
            nc.sync.dma_start(s[:], t[:])
            return s

        w_in_aT_s = cpool.tile([128, len(KA), H], BF16, tag="w_in_aT")
        for ki, (k0, kn) in enumerate(KA):
            nc.sync.dma_start(w_in_aT_s[0:kn, ki, :], w_in_aT[k0 : k0 + kn, :])
        b_in_a_s = load_const(b_in_a, [H, 1], FP32, "b_in_a")
        w_in_uT_s = load_const(w_in_uT, [DU, H], BF16, "w_in_uT")
        b_in_u_s = load_const(b_in_u, [H, 1], FP32, "b_in_u")
        convw_s = {}
        for et in ("c1p", "c1b", "c2p"):
            wlT, bl, wrT = convw[et]
            convw_s[et] = (
                load_const(wlT, [H, H], BF16, f"{et}_wlT"),
                load_const(bl, [H, 1], FP32, f"{et}_bl"),
                load_const(wrT, [H, H], BF16, f"{et}_wrT"),
            )
        w_outT_s = load_const(w_outT, [H, OUT], BF16, "w_outT")
        b_out_s = load_const(b_out, [OUT, 1], FP32, "b_out")
        idx_p_s = load_const(idx_p, [128, NBLK * 4 * CQp * 8], I16, "idx_p")
        slot_p_s = load_const(slot_p, [128, NBLK * CBp], FP32, "slot_p")
        rval_p_s = load_const(rval_p, [128, NBLK * CBp], FP32, "rval_p")
        slot_b_s = load_const(slot_b, [128, NBLK * CBb], FP32, "slot_b")
        rval_b_s = load_const(rval_b, [128, NBLK * CBb], FP32, "rval_b")

        # resident feature-major node tables (own shard)
        uT_own = cpool.tile([H, shard], BF16, tag="uT_own")
        aT_own = cpool.tile([H, shard], BF16, tag="aT_own")
        a1T = cpool.tile([H, shard], BF16, tag="a1T")

        def transpose_out(pool_ps, pool_st, src_ap, b, shard_dram):
            """[64,128] feature-major block -> [128,64] -> shard_dram rows."""
            tp = pool_ps.tile([128, H], BF16, tag="tpps")
            nc.tensor.transpose(tp[:], src_ap, ident_sb[0:H, 0:H])
            st = pool_st.tile([128, H], BF16, tag="tpst")
            nc.scalar.copy(st[:], tp[:])
            nc.sync.dma_start(shard_dram[b * 128 : (b + 1) * 128, 0:H], st[:])

        # ------------------- stage 1: input projections -------------------
        def _inproj():
          with (
            tc.tile_pool(name="ip_ps", bufs=3, space="PSUM") as ip_ps,
            tc.tile_pool(name="tp_ps", bufs=2, space="PSUM") as tp_ps,
            tc.tile_pool(name="ip_sb", bufs=4) as ip_sb,
            tc.tile_pool(name="tp_sb", bufs=3) as tp_sb,
        ):
            for t0, tw in n_tw:
                xt = ip_sb.tile([DU, TW], BF16, tag="xu")
                nc.sync.dma_start(xt[:, 0:tw], xuT[:, t0 : t0 + tw])
                ps = ip_ps.tile([H, TW], FP32, tag="ipps")
                nc.tensor.matmul(ps[:, 0:tw], w_in_uT_s[:], xt[:, 0:tw])
                nc.scalar.activation(
                    uT_own[:, t0 : t0 + tw], ps[:, 0:tw], AF.Relu, bias=b_in_u_s[:]
                )
            for t0, tw in n_tw:
                ps = ip_ps.tile([H, TW], FP32, tag="ipps")
                for ki, (k0, kn) in enumerate(KA):
                    xt = ip_sb.tile([128, TW], BF16, tag="xa")
                    nc.sync.dma_start(xt[0:kn, 0:tw], xaT[k0 : k0 + kn, t0 : t0 + tw])
                    nc.tensor.matmul(
                        ps[:, 0:tw],
                        w_in_aT_s[0:kn, ki, :],
                        xt[0:kn, 0:tw],
                        start=(ki == 0),
                        stop=(ki == len(KA) - 1),
                    )
                nc.scalar.activation(
                    aT_own[:, t0 : t0 + tw], ps[:, 0:tw], AF.Relu, bias=b_in_a_s[:]
                )
            for b in range(NBLK):
                transpose_out(tp_ps, tp_sb, uT_own[:, b * 128 : (b + 1) * 128], b, u_shard)
                transpose_out(tp_ps, tp_sb, aT_own[:, b * 128 : (b + 1) * 128], b, a_shard)

        # ------------------- all-gather u, a -------------------
        def _ag_ua():
            nc.gpsimd.collective_compute(
                "AllGather", ALU.bypass, replica_groups=groups,
                ins=[u_shard[:]], outs=[u_rm[:]],
            )
            nc.gpsimd.collective_compute(
                "AllGather", ALU.bypass, replica_groups=groups,
                ins=[a_shard[:]], outs=[a_rm[:]],
            )

        # ------------------- conv layers -------------------
        def conv_layer(
            pools, gtable, idx_res, idx_dram, slot_s, rval_s, CQ, et, xdstT,
            outT, relu, shard_dram, head,
        ):
            CB = 4 * CQ
            (msg_p, s_p, agg_ps, lin_ps, agg_sb, ctp_ps, ctp_sb, outb_p,
             idx_pool, hd_ps, hd_sb) = pools
            wlT_s, bl_s, wrT_s = convw_s[et]
            for b in range(NBLK):
                if idx_res is not None:
                    idxt = idx_res[:, b * 4 * CQ * 8 : (b + 1) * 4 * CQ * 8]
                else:
                    it = idx_pool.tile([128, 4 * CQ * 8], I16, tag="idxs")
                    nc.sync.dma_start(
                        it[:], idx_dram[:, b * 4 * CQ * 8 : (b + 1) * 4 * CQ * 8]
                    )
                    idxt = it[:]
                msg = msg_p.tile([128, CB, 128], BF16, tag="msg")
                for q in range(4):
                    nc.gpsimd.dma_gather(
                        msg[:, q * CQ : (q + 1) * CQ, :],
                        gtable[q * QN : (q + 1) * QN, :],
                        idxt[:, q * CQ * 8 : (q + 1) * CQ * 8],
                        CQ * 128,
                        CQ * 128,
                        128,
                    )
                agg = agg_ps.tile([H, 128], FP32, tag="agg")
                for c in range(CB):
                    S = s_p.tile([128, 128], BF16, tag="S")
                    nc.vector.tensor_scalar(
                        S[:],
                        iota_sb[:],
                        slot_s[:, b * CB + c : b * CB + c + 1],
                        rval_s[:, b * CB + c : b * CB + c + 1],
                        ALU.is_equal,
                        ALU.mult,
                    )
                    nc.tensor.matmul(
                        agg[:],
                        msg[:, c, 0:H],
                        S[:],
                        start=(c == 0),
                        stop=(c == CB - 1),
                    )
                aggs = agg_sb.tile([H, 128], BF16, tag="aggs")
                nc.scalar.copy(aggs[:], agg[:])
                lin = lin_ps.tile([H, 128], FP32, tag="lin")
                nc.tensor.matmul(lin[:], wlT_s[:], aggs[:], start=True, stop=False)
                nc.tensor.matmul(
                    lin[:],
                    wrT_s[:],
                    xdstT[:, b * 128 : (b + 1) * 128],
                    start=False,
                    stop=True,
                )
                if outT is not None:
                    ovec = outT[:, b * 128 : (b + 1) * 128]
                else:
                    ob = outb_p.tile([H, 128], BF16, tag="outb")
                    ovec = ob[:]
                if relu:
                    nc.scalar.activation(ovec, lin[:], AF.Relu, bias=bl_s[:])
                else:
                    nc.vector.tensor_scalar_add(ovec, lin[:], bl_s[:])
                if shard_dram is not None:
                    transpose_out(ctp_ps, ctp_sb, ovec, b, shard_dram)
                if head:
                    hp = hd_ps.tile([OUT, 128], FP32, tag="hdps")
                    nc.tensor.matmul(hp[:], w_outT_s[:], ovec)
                    ho = hd_sb.tile([OUT, 128], FP32, tag="hdo")
                    nc.vector.tensor_scalar_add(ho[:], hp[:], b_out_s[:])
                    nc.sync.dma_start(out_d[:, b * 128 : (b + 1) * 128], ho[:])

        def _convs():
          with (
            tc.tile_pool(name="msg", bufs=3) as msg_p,
            tc.tile_pool(name="S", bufs=4) as s_p,
            tc.tile_pool(name="agg_ps", bufs=2, space="PSUM") as agg_ps,
            tc.tile_pool(name="lin_ps", bufs=2, space="PSUM") as lin_ps,
            tc.tile_pool(name="agg_sb", bufs=3) as agg_sb,
            tc.tile_pool(name="ctp_ps", bufs=2, space="PSUM") as ctp_ps,
            tc.tile_pool(name="ctp_sb", bufs=3) as ctp_sb,
            tc.tile_pool(name="outb", bufs=3) as outb_p,
            tc.tile_pool(name="idxs", bufs=3) as idx_pool,
            tc.tile_pool(name="hd_ps", bufs=2, space="PSUM") as hd_ps,
            tc.tile_pool(name="hd_sb", bufs=3) as hd_sb,
        ):
            pools = (msg_p, s_p, agg_ps, lin_ps, agg_sb, ctp_ps, ctp_sb,
                     outb_p, idx_pool, hd_ps, hd_sb)
            # users first so the u1 all-gather overlaps the articles conv
            conv_layer(
                pools, a_rm, None, idx_b, slot_b_s, rval_b_s, CQb, "c1b",
                uT_own, None, True, u1_shard, False,
            )
            nc.gpsimd.collective_compute(
                "AllGather", ALU.bypass, replica_groups=groups,
                ins=[u1_shard[:]], outs=[u1_rm[:]],
            )
            conv_layer(
                pools, u_rm, idx_p_s, None, slot_p_s, rval_p_s, CQp, "c1p",
                aT_own, a1T, True, None, False,
            )
            conv_layer(
                pools, u1_rm, idx_p_s, None, slot_p_s, rval_p_s, CQp, "c2p",
                a1T, None, False, None, True,
            )

        for _rep in range(reps):
            _inproj()
            _ag_ua()
            _convs()

    nc.compile()
    return nc


# ----------------------------------------------------------------------------
# Entry point
# ----------------------------------------------------------------------------

_CACHE = {}


def _run(inputs, cfg, trace=False, reps=1):
    N, DA, DU, H = cfg["N"], cfg["DA"], cfg["DU"], cfg["H"]
    n_cores, shard = cfg["n_cores"], cfg["shard"]

    CQp, per_core_p = prep_edges(inputs["ei_posts"][0], inputs["ei_posts"][1], cfg)
    CQb, per_core_b = prep_edges(inputs["ei_pb"][0], inputs["ei_pb"][1], cfg)

    key = (tuple(sorted(cfg.items())), CQp, CQb, reps)
    if key not in _CACHE:
        _CACHE[key] = build_program(cfg, CQp, CQb, reps)
    nc = _CACHE[key]

    DA_PAD = ((DA + 15) // 16) * 16
    xa = np.asarray(inputs["x_article"], np.float32)
    xu = np.asarray(inputs["x_user"], np.float32)

    shared = dict(
        w_in_aT=np.concatenate(
            [_lin_bf16(inputs["w_in_a"]), np.zeros((DA_PAD - DA, H), BF16_NP)], 0
        ),
        b_in_a=_bias_col(inputs["b_in_a"]),
        w_in_uT=_lin_bf16(inputs["w_in_u"]),
        b_in_u=_bias_col(inputs["b_in_u"]),
        w_outT=_lin_bf16(inputs["w_out"]),
        b_out=_bias_col(inputs["b_out"]),
        iota=np.tile(np.arange(128, dtype=np.float32), (128, 1)),
        ident=np.eye(128, dtype=BF16_NP),
    )
    for et, pfx in (("c1p", "c1p"), ("c1b", "c1b"), ("c2p", "c2p")):
        shared[f"{et}_wlT"] = _lin_bf16(inputs[f"{pfx}_wl"])
        shared[f"{et}_bl"] = _bias_col(inputs[f"{pfx}_bl"])
        shared[f"{et}_wrT"] = _lin_bf16(inputs[f"{pfx}_wr"])

    in_maps = []
    for c in range(n_cores):
        c0, c1 = c * shard, min((c + 1) * shard, N)
        xaT_c = np.zeros((DA_PAD, shard), BF16_NP)
        xaT_c[:DA, : c1 - c0] = xa[c0:c1].T.astype(BF16_NP)
        xuT_c = np.zeros((DU, shard), BF16_NP)
        xuT_c[:, : c1 - c0] = xu[c0:c1].T.astype(BF16_NP)
        m = dict(shared)
        m["xaT"] = xaT_c
        m["xuT"] = xuT_c
        m["idx_p"] = per_core_p[c]["idx_w"]
        m["slot_p"] = per_core_p[c]["slot_w"]
        m["rval_p"] = per_core_p[c]["rval_w"]
        m["idx_b"] = per_core_b[c]["idx_w"]
        m["slot_b"] = per_core_b[c]["slot_w"]
        m["rval_b"] = per_core_b[c]["rval_w"]
        in_maps.append(m)

    res = run_bass_kernel_spmd(nc, in_maps, list(range(n_cores)), trace=trace)
    outs = [res.results[c]["out"] for c in range(n_cores)]  # [2, shard] each
    full = np.concatenate(outs, axis=1)[:, :N].T.astype(np.float32)
    return np.ascontiguousarray(full), res


def kernel(**inputs):
    out, _ = _run(inputs, full_cfg(), trace=False)
    return out


# revision 18
# speedup vs baseline: 20.3629x; 20.3629x over previous
"""Trainium2 Bass kernel for hetero GNN (2x SAGEConv layers + in/out proj).

Full inputs in, full output out. Internally: dst-node sharding across 8
NeuronCores, edge bucketing by (dst block of 128, src quadrant) on host,
device-side gather via SWDGE dma_gather, segment-mean via one-hot matmul
accumulated in PSUM, AllGather collectives for the shared node tables.
"""

import math

import numpy as np

import concourse.bacc as bacc
import concourse.bass as bass
import concourse.mybir as mybir
from concourse import tile
from concourse.bass_utils import run_bass_kernel_spmd

FP32 = mybir.dt.float32
BF16 = mybir.dt.bfloat16
I16 = mybir.dt.int16
AF = mybir.ActivationFunctionType
ALU = mybir.AluOpType

BF16_NP = mybir.dt.np(BF16)


def full_cfg():
    return dict(
        N=100000,
        E=1600000,
        DA=300,
        DU=64,
        H=64,
        OUT=2,
        n_cores=8,
        shard=12544,  # 98 * 128 per-core dst shard
        cq_min=5,
    )


# ----------------------------------------------------------------------------
# Host-side edge preprocessing
# ----------------------------------------------------------------------------


def prep_edges(src, dst, cfg):
    """Bucket edges by (dst block of 128, src quadrant); build gather index /
    one-hot slot / reciprocal-degree arrays per core.

    Returns (CQ, per_core list of dicts with idx_w/slot_w/rval_w).
    """
    N, shard, n_cores = cfg["N"], cfg["shard"], cfg["n_cores"]
    NPAD = n_cores * shard
    QN = NPAD // 4
    assert QN < 32768, QN
    NBLK = shard // 128

    src = np.asarray(src, dtype=np.int64)
    dst = np.asarray(dst, dtype=np.int64)
    deg = np.bincount(dst, minlength=N).astype(np.float64)
    recip = (1.0 / np.maximum(deg, 1.0)).astype(np.float32)

    blk = dst >> 7  # global 128-block id
    quad = src // QN
    n_cells = n_cores * NBLK * 4
    cell = blk * 4 + quad
    # sort edges by cell (order within a cell is irrelevant)
    order = np.argsort(cell, kind="stable")
    c_src = src[order]
    c_dst = dst[order]
    c_cell = cell[order]
    starts = np.searchsorted(c_cell, np.arange(n_cells))
    cnts = np.bincount(c_cell, minlength=n_cells)
    CQ = max(cfg["cq_min"], int(math.ceil(cnts.max() / 128)))
    CB = 4 * CQ

    j = np.arange(len(c_src)) - starts[c_cell]  # position within cell
    loc_idx = (c_src - quad[order] * QN).astype(np.int16)
    slot_val = (c_dst & 127).astype(np.float32)
    rval_val = recip[c_dst]

    b_local_all = (c_cell // 4) % NBLK
    q_all = c_cell % 4
    core_all = c_cell // (4 * NBLK)

    per_core = []
    for c in range(n_cores):
        m = core_all == c
        bl = b_local_all[m]
        q = q_all[m]
        jj = j[m]
        # gather idx array, 16-partition wrapped, replicated 8x
        idx_w = np.zeros((128, NBLK * 4 * CQ * 8), dtype=np.int16)
        col = (bl * 4 + q) * (CQ * 8) + jj // 16
        row = jj % 16
        for g in range(8):
            idx_w[row + 16 * g, col] = loc_idx[m]
        # slot / recip-val arrays: [128, NBLK*CB]
        slot_w = np.full((128, NBLK * CB), 999.0, dtype=np.float32)
        rval_w = np.zeros((128, NBLK * CB), dtype=np.float32)
        colS = bl * CB + q * CQ + jj // 128
        rowS = jj % 128
        slot_w[rowS, colS] = slot_val[m]
        rval_w[rowS, colS] = rval_val[m]
        per_core.append(dict(idx_w=idx_w, slot_w=slot_w, rval_w=rval_w))
    return CQ, per_core


def _lin_bf16(w):
    """[out,in] fp32 -> lhsT layout [in,out] bf16."""
    return np.ascontiguousarray(w.T).astype(BF16_NP)


def _bias_col(b):
    return np.asarray(b, np.float32).reshape(-1, 1)


# ----------------------------------------------------------------------------
# Device program
# ----------------------------------------------------------------------------


def build_program(cfg, CQp, CQb, reps=1):
    N, DA, DU, H, OUT = cfg["N"], cfg["DA"], cfg["DU"], cfg["H"], cfg["OUT"]
    n_cores, shard = cfg["n_cores"], cfg["shard"]
    NPAD = n_cores * shard
    QN = NPAD // 4
    NBLK = shard // 128
    CBp, CBb = 4 * CQp, 4 * CQb
    DA_PAD = ((DA + 15) // 16) * 16  # 304
    KA = [(k, min(128, DA_PAD - k)) for k in range(0, DA_PAD, 128)]
    TW = 512  # in-proj / head tile width
    n_tw = [(t, min(TW, shard - t)) for t in range(0, shard, TW)]

    nc = bacc.Bacc("TRN2", debug=False)

    # ---- I/O ----
    xaT = nc.dram_tensor("xaT", [DA_PAD, shard], BF16, kind="ExternalInput")
    xuT = nc.dram_tensor("xuT", [DU, shard], BF16, kind="ExternalInput")
    w_in_aT = nc.dram_tensor("w_in_aT", [DA_PAD, H], BF16, kind="ExternalInput")
    b_in_a = nc.dram_tensor("b_in_a", [H, 1], FP32, kind="ExternalInput")
    w_in_uT = nc.dram_tensor("w_in_uT", [DU, H], BF16, kind="ExternalInput")
    b_in_u = nc.dram_tensor("b_in_u", [H, 1], FP32, kind="ExternalInput")
    convw = {}
    for et in ("c1p", "c1b", "c2p"):
        convw[et] = (
            nc.dram_tensor(f"{et}_wlT", [H, H], BF16, kind="ExternalInput"),
            nc.dram_tensor(f"{et}_bl", [H, 1], FP32, kind="ExternalInput"),
            nc.dram_tensor(f"{et}_wrT", [H, H], BF16, kind="ExternalInput"),
        )
    w_outT = nc.dram_tensor("w_outT", [H, OUT], BF16, kind="ExternalInput")
    b_out = nc.dram_tensor("b_out", [OUT, 1], FP32, kind="ExternalInput")
    iota_in = nc.dram_tensor("iota", [128, 128], FP32, kind="ExternalInput")
    ident_in = nc.dram_tensor("ident", [128, 128], BF16, kind="ExternalInput")
    idx_p = nc.dram_tensor("idx_p", [128, NBLK * 4 * CQp * 8], I16, kind="ExternalInput")
    slot_p = nc.dram_tensor("slot_p", [128, NBLK * CBp], FP32, kind="ExternalInput")
    rval_p = nc.dram_tensor("rval_p", [128, NBLK * CBp], FP32, kind="ExternalInput")
    idx_b = nc.dram_tensor("idx_b", [128, NBLK * 4 * CQb * 8], I16, kind="ExternalInput")
    slot_b = nc.dram_tensor("slot_b", [128, NBLK * CBb], FP32, kind="ExternalInput")
    rval_b = nc.dram_tensor("rval_b", [128, NBLK * CBb], FP32, kind="ExternalInput")
    out_d = nc.dram_tensor("out", [OUT, shard], FP32, kind="ExternalOutput")

    # internal HBM
    u_shard = nc.dram_tensor("u_shard", [shard, 128], BF16)
    a_shard = nc.dram_tensor("a_shard", [shard, 128], BF16)
    u1_shard = nc.dram_tensor("u1_shard", [shard, 128], BF16)
    u_rm = nc.dram_tensor("u_rm", [NPAD, 128], BF16, addr_space="Shared")
    a_rm = nc.dram_tensor("a_rm", [NPAD, 128], BF16, addr_space="Shared")
    u1_rm = nc.dram_tensor("u1_rm", [NPAD, 128], BF16, addr_space="Shared")
    groups = [list(range(n_cores))]

    from contextlib import ExitStack

    with tile.TileContext(nc) as tc, ExitStack() as _stack:
        cpool = _stack.enter_context(tc.tile_pool(name="const", bufs=1))
        # resident constants
        iota_sb = cpool.tile([128, 128], FP32, tag="iota")
        ident_sb = cpool.tile([128, 128], BF16, tag="ident")
        nc.sync.dma_start(iota_sb[:], iota_in[:])
        nc.sync.dma_start(ident_sb[:], ident_in[:])

        def load_const(t, shape, dtype, tag):
            s = cpool.tile(shape, dtype, tag=tag)
            nc.sync.dma_start(s[:], t[:])
            return s

        w_in_aT_s = cpool.tile([128, len(KA), H], BF16, tag="w_in_aT")
        for ki, (k0, kn) in enumerate(KA):
            nc.sync.dma_start(w_in_aT_s[0:kn, ki, :], w_in_aT[k0 : k0 + kn, :])
        b_in_a_s = load_const(b_in_a, [H, 1], FP32, "b_in_a")
        w_in_uT_s = load_const(w_in_uT, [DU, H], BF16, "w_in_uT")
        b_in_u_s = load_const(b_in_u, [H, 1], FP32, "b_in_u")
        convw_s = {}
        for et in ("c1p", "c1b", "c2p"):
            wlT, bl, wrT = convw[et]
            convw_s[et] = (
                load_const(wlT, [H, H], BF16, f"{et}_wlT"),
                load_const(bl, [H, 1], FP32, f"{et}_bl"),
                load_const(wrT, [H, H], BF16, f"{et}_wrT"),
            )
        w_outT_s = load_const(w_outT, [H, OUT], BF16, "w_outT")
        b_out_s = load_const(b_out, [OUT, 1], FP32, "b_out")
        idx_p_s = load_const(idx_p, [128, NBLK * 4 * CQp * 8], I16, "idx_p")
        slot_p_s = load_const(slot_p, [128, NBLK * CBp], FP32, "slot_p")
        rval_p_s = load_const(rval_p, [128, NBLK * CBp], FP32, "rval_p")
        slot_b_s = load_const(slot_b, [128, NBLK * CBb], FP32, "slot_b")
        rval_b_s = load_const(rval_b, [128, NBLK * CBb], FP32, "rval_b")

        # resident feature-major node tables (own shard)
        uT_own = cpool.tile([H, shard], BF16, tag="uT_own")
        aT_own = cpool.tile([H, shard], BF16, tag="aT_own")
        a1T = cpool.tile([H, shard], BF16, tag="a1T")

        def transpose_out(pool_ps, pool_st, src_ap, b, shard_dram):
            """[64,128] feature-major block -> [128,64] -> shard_dram rows."""
            tp = pool_ps.tile([128, H], BF16, tag="tpps")
            nc.tensor.transpose(tp[:], src_ap, ident_sb[0:H, 0:H])
            st = pool_st.tile([128, H], BF16, tag="tpst")
            nc.scalar.copy(st[:], tp[:])
            nc.sync.dma_start(shard_dram[b * 128 : (b + 1) * 128, 0:H], st[:])

        # ------------------- stage 1: input projections -------------------
        def _inproj():
          with (
            tc.tile_pool(name="ip_ps", bufs=3, space="PSUM") as ip_ps,
            tc.tile_pool(name="tp_ps", bufs=2, space="PSUM") as tp_ps,
            tc.tile_pool(name="ip_sb", bufs=4) as ip_sb,
            tc.tile_pool(name="tp_sb", bufs=3) as tp_sb,
        ):
            for t0, tw in n_tw:
                xt = ip_sb.tile([DU, TW], BF16, tag="xu")
                nc.sync.dma_start(xt[:, 0:tw], xuT[:, t0 : t0 + tw])
                ps = ip_ps.tile([H, TW], FP32, tag="ipps")
                nc.tensor.matmul(ps[:, 0:tw], w_in_uT_s[:], xt[:, 0:tw])
                nc.scalar.activation(
                    uT_own[:, t0 : t0 + tw], ps[:, 0:tw], AF.Relu, bias=b_in_u_s[:]
                )
            for t0, tw in n_tw:
                ps = ip_ps.tile([H, TW], FP32, tag="ipps")
                for ki, (k0, kn) in enumerate(KA):
                    xt = ip_sb.tile([128, TW], BF16, tag="xa")
                    nc.sync.dma_start(xt[0:kn, 0:tw], xaT[k0 : k0 + kn, t0 : t0 + tw])
                    nc.tensor.matmul(
                        ps[:, 0:tw],
                        w_in_aT_s[0:kn, ki, :],
                        xt[0:kn, 0:tw],
                        start=(ki == 0),
                        stop=(ki == len(KA) - 1),
                    )
                nc.scalar.activation(
                    aT_own[:, t0 : t0 + tw], ps[:, 0:tw], AF.Relu, bias=b_in_a_s[:]
                )
            for b in range(NBLK):
                transpose_out(tp_ps, tp_sb, uT_own[:, b * 128 : (b + 1) * 128], b, u_shard)
                transpose_out(tp_ps, tp_sb, aT_own[:, b * 128 : (b + 1) * 128], b, a_shard)

        # ------------------- all-gather u, a -------------------
        def _ag_ua():
            nc.gpsimd.collective_compute(
                "AllGather", ALU.bypass, replica_groups=groups,
                ins=[u_shard[:]], outs=[u_rm[:]],
            )
            nc.gpsimd.collective_compute(
                "AllGather", ALU.bypass, replica_groups=groups,
                ins=[a_shard[:]], outs=[a_rm[:]],
            )

        # ------------------- conv layers -------------------
        def conv_layer(
            pools, gtable, idx_res, idx_dram, slot_s, rval_s, CQ, et, xdstT,
            outT, relu, shard_dram, head,
        ):
            CB = 4 * CQ
            (msg_p, s_p, agg_ps, lin_ps, agg_sb, ctp_ps, ctp_sb, outb_p,
             idx_pool, hd_ps, hd_sb) = pools
            wlT_s, bl_s, wrT_s = convw_s[et]
            for b in range(NBLK):
                if idx_res is not None:
                    idxt = idx_res[:, b * 4 * CQ * 8 : (b + 1) * 4 * CQ * 8]
                else:
                    it = idx_pool.tile([128, 4 * CQ * 8], I16, tag="idxs")
                    nc.sync.dma_start(
                        it[:], idx_dram[:, b * 4 * CQ * 8 : (b + 1) * 4 * CQ * 8]
                    )
                    idxt = it[:]
                msg = msg_p.tile([128, CB, 128], BF16, tag="msg")
                for q in range(4):
                    nc.gpsimd.dma_gather(
                        msg[:, q * CQ : (q + 1) * CQ, :],
                        gtable[q * QN : (q + 1) * QN, :],
                        idxt[:, q * CQ * 8 : (q + 1) * CQ * 8],
                        CQ * 128,
                        CQ * 128,
                        128,
                    )
                agg = agg_ps.tile([H, 128], FP32, tag="agg")
                for c in range(CB):
                    S = s_p.tile([128, 128], BF16, tag="S")
                    nc.vector.tensor_scalar(
                        S[:],
                        iota_sb[:],
                        slot_s[:, b * CB + c : b * CB + c + 1],
                        rval_s[:, b * CB + c : b * CB + c + 1],
                        ALU.is_equal,
                        ALU.mult,
                    )
                    nc.tensor.matmul(
                        agg[:],
                        msg[:, c, 0:H],
                        S[:],
                        start=(c == 0),
                        stop=(c == CB - 1),
                    )
                aggs = agg_sb.tile([H, 128], BF16, tag="aggs")
                nc.scalar.copy(aggs[:], agg[:])
                lin = lin_ps.tile([H, 128], FP32, tag="lin")
                nc.tensor.matmul(lin[:], wlT_s[:], aggs[:], start=True, stop=False)
                nc.tensor.matmul(
                    lin[:],
                    wrT_s[:],
                    xdstT[:, b * 128 : (b + 1) * 128],
                    start=False,
                    stop=True,
                )
                if outT is not None:
                    ovec = outT[:, b * 128 : (b + 1) * 128]
                else:
                    ob = outb_p.tile([H, 128], BF16, tag="outb")
                    ovec = ob[:]
                if relu:
                    nc.scalar.activation(ovec, lin[:], AF.Relu, bias=bl_s[:])
                else:
                    nc.vector.tensor_scalar_add(ovec, lin[:], bl_s[:])
                if shard_dram is not None:
                    transpose_out(ctp_ps, ctp_sb, ovec, b, shard_dram)
                if head:
                    hp = hd_ps.tile([OUT, 128], FP32, tag="hdps")
                    nc.tensor.matmul(hp[:], w_outT_s[:], ovec)
                    ho = hd_sb.tile([OUT, 128], FP32, tag="hdo")
                    nc.vector.tensor_scalar_add(ho[:], hp[:], b_out_s[:])
                    nc.sync.dma_start(out_d[:, b * 128 : (b + 1) * 128], ho[:])

        def _convs():
          with (
            tc.tile_pool(name="msg", bufs=3) as msg_p,
            tc.tile_pool(name="S", bufs=4) as s_p,
            tc.tile_pool(name="agg_ps", bufs=2, space="PSUM") as agg_ps,
            tc.tile_pool(name="lin_ps", bufs=2, space="PSUM") as lin_ps,
            tc.tile_pool(name="agg_sb", bufs=3) as agg_sb,
            tc.tile_pool(name="ctp_ps", bufs=2, space="PSUM") as ctp_ps,
            tc.tile_pool(name="ctp_sb", bufs=3) as ctp_sb,
            tc.tile_pool(name="outb", bufs=3) as outb_p,
            tc.tile_pool(name="idxs", bufs=3) as idx_pool,
            tc.tile_pool(name="hd_ps", bufs=2, space="PSUM") as hd_ps,
            tc.tile_pool(name="hd_sb", bufs=3) as hd_sb,
        ):
            pools = (msg_p, s_p, agg_ps, lin_ps, agg_sb, ctp_ps, ctp_sb,
                     outb_p, idx_pool, hd_ps, hd_sb)
            # users first so the u1 all-gather overlaps the articles conv
            conv_layer(
                pools, a_rm, None, idx_b, slot_b_s, rval_b_s, CQb, "c1b",
                uT_own, None, True, u1_shard, False,
            )
            nc.gpsimd.collective_compute(
                "AllGather", ALU.bypass, replica_groups=groups,
                ins=[u1_shard[:]], outs=[u1_rm[:]],
            )
            conv_layer(
                pools, u_rm, idx_p_s, None, slot_p_s, rval_p_s, CQp, "c1p",
                aT_own, a1T, True, None, False,
            )
            conv_layer(
                pools, u1_rm, idx_p_s, None, slot_p_s, rval_p_s, CQp, "c2p",
                a1T, None, False, None, True,
            )

        for _rep in range(reps):
            _inproj()
            _ag_ua()
            _convs()

    nc.compile()
    return nc


# ----------------------------------------------------------------------------
# Entry point
# ----------------------------------------------------------------------------

_CACHE = {}


def build_in_maps(inputs, cfg, CQp, per_core_p, CQb, per_core_b):
    N, DA, DU, H = cfg["N"], cfg["DA"], cfg["DU"], cfg["H"]
    n_cores, shard = cfg["n_cores"], cfg["shard"]
    DA_PAD = ((DA + 15) // 16) * 16
    xa = np.asarray(inputs["x_article"], np.float32)
    xu = np.asarray(inputs["x_user"], np.float32)

    shared = dict(
        w_in_aT=np.concatenate(
            [_lin_bf16(inputs["w_in_a"]), np.zeros((DA_PAD - DA, H), BF16_NP)], 0
        ),
        b_in_a=_bias_col(inputs["b_in_a"]),
        w_in_uT=_lin_bf16(inputs["w_in_u"]),
        b_in_u=_bias_col(inputs["b_in_u"]),
        w_outT=_lin_bf16(inputs["w_out"]),
        b_out=_bias_col(inputs["b_out"]),
        iota=np.tile(np.arange(128, dtype=np.float32), (128, 1)),
        ident=np.eye(128, dtype=BF16_NP),
    )
    for et, pfx in (("c1p", "c1p"), ("c1b", "c1b"), ("c2p", "c2p")):
        shared[f"{et}_wlT"] = _lin_bf16(inputs[f"{pfx}_wl"])
        shared[f"{et}_bl"] = _bias_col(inputs[f"{pfx}_bl"])
        shared[f"{et}_wrT"] = _lin_bf16(inputs[f"{pfx}_wr"])

    in_maps = []
    for c in range(n_cores):
        c0, c1 = c * shard, min((c + 1) * shard, N)
        xaT_c = np.zeros((DA_PAD, shard), BF16_NP)
        xaT_c[:DA, : c1 - c0] = xa[c0:c1].T.astype(BF16_NP)
        xuT_c = np.zeros((DU, shard), BF16_NP)
        xuT_c[:, : c1 - c0] = xu[c0:c1].T.astype(BF16_NP)
        m = dict(shared)
        m["xaT"] = xaT_c
        m["xuT"] = xuT_c
        m["idx_p"] = per_core_p[c]["idx_w"]
        m["slot_p"] = per_core_p[c]["slot_w"]
        m["rval_p"] = per_core_p[c]["rval_w"]
        m["idx_b"] = per_core_b[c]["idx_w"]
        m["slot_b"] = per_core_b[c]["slot_w"]
        m["rval_b"] = per_core_b[c]["rval_w"]
        in_maps.append(m)
    return in_maps


def _run(inputs, cfg, trace=False, reps=1):
    N, n_cores, shard = cfg["N"], cfg["n_cores"], cfg["shard"]

    CQp, per_core_p = prep_edges(inputs["ei_posts"][0], inputs["ei_posts"][1], cfg)
    CQb, per_core_b = prep_edges(inputs["ei_pb"][0], inputs["ei_pb"][1], cfg)

    key = (tuple(sorted(cfg.items())), CQp, CQb, reps)
    if key not in _CACHE:
        _CACHE[key] = build_program(cfg, CQp, CQb, reps)
    nc = _CACHE[key]

    in_maps = build_in_maps(inputs, cfg, CQp, per_core_p, CQb, per_core_b)

    res = run_bass_kernel_spmd(nc, in_maps, list(range(n_cores)), trace=trace)
    outs = [res.results[c]["out"] for c in range(n_cores)]  # [2, shard] each
    full = np.concatenate(outs, axis=1)[:, :N].T.astype(np.float32)
    return np.ascontiguousarray(full), res


def kernel(**inputs):
    out, _ = _run(inputs, full_cfg(), trace=False)
    return out


# revision 23
# speedup vs baseline: 257.6064x; 12.6508x over previous
"""Trainium2 Bass kernel for hetero GNN (2x SAGEConv layers + in/out proj).

Full inputs in, full output out. Internally: dst-node sharding across 8
NeuronCores, edge bucketing by (dst block of 128, src quadrant) on host,
device-side gather via SWDGE dma_gather, segment-mean via one-hot matmul
accumulated in PSUM, AllGather collectives for the shared node tables.
"""

import math

import numpy as np

import concourse.bacc as bacc
import concourse.bass as bass
import concourse.mybir as mybir
from concourse import tile
from concourse.bass_utils import run_bass_kernel_spmd

FP32 = mybir.dt.float32
BF16 = mybir.dt.bfloat16
I16 = mybir.dt.int16
AF = mybir.ActivationFunctionType
ALU = mybir.AluOpType

BF16_NP = mybir.dt.np(BF16)


def full_cfg():
    return dict(
        N=100000,
        E=1600000,
        DA=300,
        DU=64,
        H=64,
        OUT=2,
        n_cores=8,
        shard=12544,  # 98 * 128 per-core dst shard
        cq_min=5,
    )


# ----------------------------------------------------------------------------
# Host-side edge preprocessing
# ----------------------------------------------------------------------------


def prep_edges(src, dst, cfg):
    """Bucket edges by (dst block of 128, src quadrant); build gather index /
    one-hot slot / reciprocal-degree arrays per core.

    Returns (CQ, per_core list of dicts with idx_w/slot_w/rval_w).
    """
    N, shard, n_cores = cfg["N"], cfg["shard"], cfg["n_cores"]
    NPAD = n_cores * shard
    QN = NPAD // 4
    assert QN < 32768, QN
    NBLK = shard // 128

    src = np.asarray(src, dtype=np.int64)
    dst = np.asarray(dst, dtype=np.int64)
    deg = np.bincount(dst, minlength=N).astype(np.float64)
    recip = (1.0 / np.maximum(deg, 1.0)).astype(np.float32)

    blk = dst >> 7  # global 128-block id
    quad = src // QN
    n_cells = n_cores * NBLK * 4
    cell = blk * 4 + quad
    # sort edges by cell (order within a cell is irrelevant)
    order = np.argsort(cell, kind="stable")
    c_src = src[order]
    c_dst = dst[order]
    c_cell = cell[order]
    starts = np.searchsorted(c_cell, np.arange(n_cells))
    cnts = np.bincount(c_cell, minlength=n_cells)
    CQ = max(cfg["cq_min"], int(math.ceil(cnts.max() / 128)))
    CB = 4 * CQ

    j = np.arange(len(c_src)) - starts[c_cell]  # position within cell
    loc_idx = (c_src - quad[order] * QN).astype(np.int16)
    slot_val = (c_dst & 127).astype(np.float32)
    rval_val = recip[c_dst]

    b_local_all = (c_cell // 4) % NBLK
    q_all = c_cell % 4
    core_all = c_cell // (4 * NBLK)

    per_core = []
    for c in range(n_cores):
        m = core_all == c
        bl = b_local_all[m]
        q = q_all[m]
        jj = j[m]
        # gather idx array, 16-partition wrapped, replicated 8x
        idx_w = np.zeros((128, NBLK * 4 * CQ * 8), dtype=np.int16)
        col = (bl * 4 + q) * (CQ * 8) + jj // 16
        row = jj % 16
        for g in range(8):
            idx_w[row + 16 * g, col] = loc_idx[m]
        # slot / recip-val arrays: [128, NBLK*CB]
        slot_w = np.full((128, NBLK * CB), 999.0, dtype=np.float32)
        rval_w = np.zeros((128, NBLK * CB), dtype=np.float32)
        colS = bl * CB + q * CQ + jj // 128
        rowS = jj % 128
        slot_w[rowS, colS] = slot_val[m]
        rval_w[rowS, colS] = rval_val[m]
        per_core.append(dict(idx_w=idx_w, slot_w=slot_w, rval_w=rval_w))
    return CQ, per_core


def _lin_bf16(w):
    """[out,in] fp32 -> lhsT layout [in,out] bf16."""
    return np.ascontiguousarray(w.T).astype(BF16_NP)


def _bias_col(b):
    return np.asarray(b, np.float32).reshape(-1, 1)


# ----------------------------------------------------------------------------
# Device program
# ----------------------------------------------------------------------------


def build_program(cfg, CQp, CQb, reps=1, skip=()):
    N, DA, DU, H, OUT = cfg["N"], cfg["DA"], cfg["DU"], cfg["H"], cfg["OUT"]
    n_cores, shard = cfg["n_cores"], cfg["shard"]
    NPAD = n_cores * shard
    QN = NPAD // 4
    NBLK = shard // 128
    CBp, CBb = 4 * CQp, 4 * CQb
    DA_PAD = ((DA + 15) // 16) * 16  # 304
    KA = [(k, min(128, DA_PAD - k)) for k in range(0, DA_PAD, 128)]
    TW = 512  # in-proj / head tile width
    n_tw = [(t, min(TW, shard - t)) for t in range(0, shard, TW)]

    nc = bacc.Bacc("TRN2", debug=False)

    # ---- I/O ----
    xaT = nc.dram_tensor("xaT", [DA_PAD, shard], BF16, kind="ExternalInput")
    xuT = nc.dram_tensor("xuT", [DU, shard], BF16, kind="ExternalInput")
    w_in_aT = nc.dram_tensor("w_in_aT", [DA_PAD, H], BF16, kind="ExternalInput")
    b_in_a = nc.dram_tensor("b_in_a", [H, 1], FP32, kind="ExternalInput")
    w_in_uT = nc.dram_tensor("w_in_uT", [DU, H], BF16, kind="ExternalInput")
    b_in_u = nc.dram_tensor("b_in_u", [H, 1], FP32, kind="ExternalInput")
    convw = {}
    for et in ("c1p", "c1b", "c2p"):
        convw[et] = (
            nc.dram_tensor(f"{et}_wlT", [H, H], BF16, kind="ExternalInput"),
            nc.dram_tensor(f"{et}_bl", [H, 1], FP32, kind="ExternalInput"),
            nc.dram_tensor(f"{et}_wrT", [H, H], BF16, kind="ExternalInput"),
        )
    w_outT = nc.dram_tensor("w_outT", [H, OUT], BF16, kind="ExternalInput")
    b_out = nc.dram_tensor("b_out", [OUT, 1], FP32, kind="ExternalInput")
    iota_in = nc.dram_tensor("iota", [128, 128], FP32, kind="ExternalInput")
    ident_in = nc.dram_tensor("ident", [128, 128], BF16, kind="ExternalInput")
    idx_p = nc.dram_tensor("idx_p", [128, NBLK * 4 * CQp * 8], I16, kind="ExternalInput")
    slot_p = nc.dram_tensor("slot_p", [128, NBLK * CBp], FP32, kind="ExternalInput")
    rval_p = nc.dram_tensor("rval_p", [128, NBLK * CBp], FP32, kind="ExternalInput")
    idx_b = nc.dram_tensor("idx_b", [128, NBLK * 4 * CQb * 8], I16, kind="ExternalInput")
    slot_b = nc.dram_tensor("slot_b", [128, NBLK * CBb], FP32, kind="ExternalInput")
    rval_b = nc.dram_tensor("rval_b", [128, NBLK * CBb], FP32, kind="ExternalInput")
    out_d = nc.dram_tensor("out", [OUT, shard], FP32, kind="ExternalOutput")

    # internal HBM
    u_shard = nc.dram_tensor("u_shard", [shard, 128], BF16)
    a_shard = nc.dram_tensor("a_shard", [shard, 128], BF16)
    u1_shard = nc.dram_tensor("u1_shard", [shard, 128], BF16)
    u_rm = nc.dram_tensor("u_rm", [NPAD, 128], BF16, addr_space="Shared")
    a_rm = nc.dram_tensor("a_rm", [NPAD, 128], BF16, addr_space="Shared")
    u1_rm = nc.dram_tensor("u1_rm", [NPAD, 128], BF16, addr_space="Shared")
    groups = [list(range(n_cores))]

    from contextlib import ExitStack

    with tile.TileContext(nc) as tc, ExitStack() as _stack:
        cpool = _stack.enter_context(tc.tile_pool(name="const", bufs=1))
        # resident constants
        iota_sb = cpool.tile([128, 128], FP32, tag="iota")
        ident_sb = cpool.tile([128, 128], BF16, tag="ident")
        nc.sync.dma_start(iota_sb[:], iota_in[:])
        nc.sync.dma_start(ident_sb[:], ident_in[:])

        def load_const(t, shape, dtype, tag):
            s = cpool.tile(shape, dtype, tag=tag)
            nc.sync.dma_start(s[:], t[:])
            return s

        w_in_aT_s = cpool.tile([128, len(KA), H], BF16, tag="w_in_aT")
        for ki, (k0, kn) in enumerate(KA):
            nc.sync.dma_start(w_in_aT_s[0:kn, ki, :], w_in_aT[k0 : k0 + kn, :])
        b_in_a_s = load_const(b_in_a, [H, 1], FP32, "b_in_a")
        w_in_uT_s = load_const(w_in_uT, [DU, H], BF16, "w_in_uT")
        b_in_u_s = load_const(b_in_u, [H, 1], FP32, "b_in_u")
        convw_s = {}
        for et in ("c1p", "c1b", "c2p"):
            wlT, bl, wrT = convw[et]
            convw_s[et] = (
                load_const(wlT, [H, H], BF16, f"{et}_wlT"),
                load_const(bl, [H, 1], FP32, f"{et}_bl"),
                load_const(wrT, [H, H], BF16, f"{et}_wrT"),
            )
        w_outT_s = load_const(w_outT, [H, OUT], BF16, "w_outT")
        b_out_s = load_const(b_out, [OUT, 1], FP32, "b_out")
        idx_p_s = load_const(idx_p, [128, NBLK * 4 * CQp * 8], I16, "idx_p")
        slot_p_s = load_const(slot_p, [128, NBLK * CBp], FP32, "slot_p")
        rval_p_s = load_const(rval_p, [128, NBLK * CBp], FP32, "rval_p")
        slot_b_s = load_const(slot_b, [128, NBLK * CBb], FP32, "slot_b")
        rval_b_s = load_const(rval_b, [128, NBLK * CBb], FP32, "rval_b")

        # resident feature-major node tables (own shard)
        uT_own = cpool.tile([H, shard], BF16, tag="uT_own")
        aT_own = cpool.tile([H, shard], BF16, tag="aT_own")
        a1T = cpool.tile([H, shard], BF16, tag="a1T")

        def transpose_out(pool_ps, pool_st, src_ap, b, shard_dram):
            """[64,128] feature-major block -> [128,64] -> shard_dram rows."""
            tp = pool_ps.tile([128, H], BF16, tag="tpps")
            nc.tensor.transpose(tp[:], src_ap, ident_sb[0:H, 0:H])
            st = pool_st.tile([128, H], BF16, tag="tpst")
            nc.scalar.copy(st[:], tp[:])
            nc.sync.dma_start(shard_dram[b * 128 : (b + 1) * 128, 0:H], st[:])

        # ------------------- stage 1: input projections -------------------
        def _inproj():
          with (
            tc.tile_pool(name="ip_ps", bufs=3, space="PSUM") as ip_ps,
            tc.tile_pool(name="tp_ps", bufs=2, space="PSUM") as tp_ps,
            tc.tile_pool(name="ip_sb", bufs=4) as ip_sb,
            tc.tile_pool(name="tp_sb", bufs=3) as tp_sb,
        ):
            for t0, tw in n_tw:
                xt = ip_sb.tile([DU, TW], BF16, tag="xu")
                nc.sync.dma_start(xt[:, 0:tw], xuT[:, t0 : t0 + tw])
                ps = ip_ps.tile([H, TW], FP32, tag="ipps")
                nc.tensor.matmul(ps[:, 0:tw], w_in_uT_s[:], xt[:, 0:tw])
                nc.scalar.activation(
                    uT_own[:, t0 : t0 + tw], ps[:, 0:tw], AF.Relu, bias=b_in_u_s[:]
                )
            for t0, tw in n_tw:
                ps = ip_ps.tile([H, TW], FP32, tag="ipps")
                for ki, (k0, kn) in enumerate(KA):
                    xt = ip_sb.tile([128, TW], BF16, tag="xa")
                    nc.sync.dma_start(xt[0:kn, 0:tw], xaT[k0 : k0 + kn, t0 : t0 + tw])
                    nc.tensor.matmul(
                        ps[:, 0:tw],
                        w_in_aT_s[0:kn, ki, :],
                        xt[0:kn, 0:tw],
                        start=(ki == 0),
                        stop=(ki == len(KA) - 1),
                    )
                nc.scalar.activation(
                    aT_own[:, t0 : t0 + tw], ps[:, 0:tw], AF.Relu, bias=b_in_a_s[:]
                )
            for b in range(NBLK):
                transpose_out(tp_ps, tp_sb, uT_own[:, b * 128 : (b + 1) * 128], b, u_shard)
                transpose_out(tp_ps, tp_sb, aT_own[:, b * 128 : (b + 1) * 128], b, a_shard)

        # ------------------- all-gather u, a -------------------
        def _ag_ua():
            if "ag" in skip:
                return
            nc.gpsimd.collective_compute(
                "AllGather", ALU.bypass, replica_groups=groups,
                ins=[u_shard[:]], outs=[u_rm[:]],
            )
            nc.gpsimd.collective_compute(
                "AllGather", ALU.bypass, replica_groups=groups,
                ins=[a_shard[:]], outs=[a_rm[:]],
            )

        # ------------------- conv layers -------------------
        def conv_layer(
            pools, gtable, idx_res, idx_dram, slot_s, rval_s, CQ, et, xdstT,
            outT, relu, shard_dram, head,
        ):
            CB = 4 * CQ
            (msg_p, s_p, agg_ps, lin_ps, agg_sb, ctp_ps, ctp_sb, outb_p,
             idx_pool, hd_ps, hd_sb) = pools
            wlT_s, bl_s, wrT_s = convw_s[et]
            for b in range(NBLK):
                if idx_res is not None:
                    idxt = idx_res[:, b * 4 * CQ * 8 : (b + 1) * 4 * CQ * 8]
                else:
                    it = idx_pool.tile([128, 4 * CQ * 8], I16, tag="idxs")
                    nc.sync.dma_start(
                        it[:], idx_dram[:, b * 4 * CQ * 8 : (b + 1) * 4 * CQ * 8]
                    )
                    idxt = it[:]
                msg = msg_p.tile([128, CB, 128], BF16, tag="msg")
                if "gather" not in skip:
                    for q in range(4):
                        nc.gpsimd.dma_gather(
                            msg[:, q * CQ : (q + 1) * CQ, :],
                            gtable[q * QN : (q + 1) * QN, :],
                            idxt[:, q * CQ * 8 : (q + 1) * CQ * 8],
                            CQ * 128,
                            CQ * 128,
                            128,
                        )
                elif b == 0:
                    nc.vector.memset(msg[:], 0.0)
                agg = agg_ps.tile([H, 128], FP32, tag="agg")
                for c in range(CB):
                    S = s_p.tile([128, 128], BF16, tag="S")
                    nc.vector.tensor_scalar(
                        S[:],
                        iota_sb[:],
                        slot_s[:, b * CB + c : b * CB + c + 1],
                        rval_s[:, b * CB + c : b * CB + c + 1],
                        ALU.is_equal,
                        ALU.mult,
                    )
                    nc.tensor.matmul(
                        agg[:],
                        msg[:, c, 0:H],
                        S[:],
                        start=(c == 0),
                        stop=(c == CB - 1),
                    )
                aggs = agg_sb.tile([H, 128], BF16, tag="aggs")
                nc.scalar.copy(aggs[:], agg[:])
                lin = lin_ps.tile([H, 128], FP32, tag="lin")
                nc.tensor.matmul(lin[:], wlT_s[:], aggs[:], start=True, stop=False)
                nc.tensor.matmul(
                    lin[:],
                    wrT_s[:],
                    xdstT[:, b * 128 : (b + 1) * 128],
                    start=False,
                    stop=True,
                )
                if outT is not None:
                    ovec = outT[:, b * 128 : (b + 1) * 128]
                else:
                    ob = outb_p.tile([H, 128], BF16, tag="outb")
                    ovec = ob[:]
                if relu:
                    nc.scalar.activation(ovec, lin[:], AF.Relu, bias=bl_s[:])
                else:
                    nc.vector.tensor_scalar_add(ovec, lin[:], bl_s[:])
                if shard_dram is not None:
                    transpose_out(ctp_ps, ctp_sb, ovec, b, shard_dram)
                if head:
                    hp = hd_ps.tile([OUT, 128], FP32, tag="hdps")
                    nc.tensor.matmul(hp[:], w_outT_s[:], ovec)
                    ho = hd_sb.tile([OUT, 128], FP32, tag="hdo")
                    nc.vector.tensor_scalar_add(ho[:], hp[:], b_out_s[:])
                    nc.sync.dma_start(out_d[:, b * 128 : (b + 1) * 128], ho[:])

        def _convs():
          with (
            tc.tile_pool(name="msg", bufs=3) as msg_p,
            tc.tile_pool(name="S", bufs=4) as s_p,
            tc.tile_pool(name="agg_ps", bufs=2, space="PSUM") as agg_ps,
            tc.tile_pool(name="lin_ps", bufs=2, space="PSUM") as lin_ps,
            tc.tile_pool(name="agg_sb", bufs=3) as agg_sb,
            tc.tile_pool(name="ctp_ps", bufs=2, space="PSUM") as ctp_ps,
            tc.tile_pool(name="ctp_sb", bufs=3) as ctp_sb,
            tc.tile_pool(name="outb", bufs=3) as outb_p,
            tc.tile_pool(name="idxs", bufs=3) as idx_pool,
            tc.tile_pool(name="hd_ps", bufs=2, space="PSUM") as hd_ps,
            tc.tile_pool(name="hd_sb", bufs=3) as hd_sb,
        ):
            pools = (msg_p, s_p, agg_ps, lin_ps, agg_sb, ctp_ps, ctp_sb,
                     outb_p, idx_pool, hd_ps, hd_sb)
            # users first so the u1 all-gather overlaps the articles conv
            conv_layer(
                pools, a_rm, None, idx_b, slot_b_s, rval_b_s, CQb, "c1b",
                uT_own, None, True, u1_shard, False,
            )
            if "ag" not in skip:
                nc.gpsimd.collective_compute(
                    "AllGather", ALU.bypass, replica_groups=groups,
                    ins=[u1_shard[:]], outs=[u1_rm[:]],
                )
            conv_layer(
                pools, u_rm, idx_p_s, None, slot_p_s, rval_p_s, CQp, "c1p",
                aT_own, a1T, True, None, False,
            )
            conv_layer(
                pools, u1_rm, idx_p_s, None, slot_p_s, rval_p_s, CQp, "c2p",
                a1T, None, False, None, True,
            )

        for _rep in range(reps):
            _inproj()
            _ag_ua()
            if "convs" not in skip:
                _convs()

    nc.compile()
    return nc


# ----------------------------------------------------------------------------
# Entry point
# ----------------------------------------------------------------------------

_CACHE = {}


def build_in_maps(inputs, cfg, CQp, per_core_p, CQb, per_core_b):
    N, DA, DU, H = cfg["N"], cfg["DA"], cfg["DU"], cfg["H"]
    n_cores, shard = cfg["n_cores"], cfg["shard"]
    DA_PAD = ((DA + 15) // 16) * 16
    xa = np.asarray(inputs["x_article"], np.float32)
    xu = np.asarray(inputs["x_user"], np.float32)

    shared = dict(
        w_in_aT=np.concatenate(
            [_lin_bf16(inputs["w_in_a"]), np.zeros((DA_PAD - DA, H), BF16_NP)], 0
        ),
        b_in_a=_bias_col(inputs["b_in_a"]),
        w_in_uT=_lin_bf16(inputs["w_in_u"]),
        b_in_u=_bias_col(inputs["b_in_u"]),
        w_outT=_lin_bf16(inputs["w_out"]),
        b_out=_bias_col(inputs["b_out"]),
        iota=np.tile(np.arange(128, dtype=np.float32), (128, 1)),
        ident=np.eye(128, dtype=BF16_NP),
    )
    for et, pfx in (("c1p", "c1p"), ("c1b", "c1b"), ("c2p", "c2p")):
        shared[f"{et}_wlT"] = _lin_bf16(inputs[f"{pfx}_wl"])
        shared[f"{et}_bl"] = _bias_col(inputs[f"{pfx}_bl"])
        shared[f"{et}_wrT"] = _lin_bf16(inputs[f"{pfx}_wr"])

    in_maps = []
    for c in range(n_cores):
        c0, c1 = c * shard, min((c + 1) * shard, N)
        xaT_c = np.zeros((DA_PAD, shard), BF16_NP)
        xaT_c[:DA, : c1 - c0] = xa[c0:c1].T.astype(BF16_NP)
        xuT_c = np.zeros((DU, shard), BF16_NP)
        xuT_c[:, : c1 - c0] = xu[c0:c1].T.astype(BF16_NP)
        m = dict(shared)
        m["xaT"] = xaT_c
        m["xuT"] = xuT_c
        m["idx_p"] = per_core_p[c]["idx_w"]
        m["slot_p"] = per_core_p[c]["slot_w"]
        m["rval_p"] = per_core_p[c]["rval_w"]
        m["idx_b"] = per_core_b[c]["idx_w"]
        m["slot_b"] = per_core_b[c]["slot_w"]
        m["rval_b"] = per_core_b[c]["rval_w"]
        in_maps.append(m)
    return in_maps


def _run(inputs, cfg, trace=False, reps=1):
    N, n_cores, shard = cfg["N"], cfg["n_cores"], cfg["shard"]

    CQp, per_core_p = prep_edges(inputs["ei_posts"][0], inputs["ei_posts"][1], cfg)
    CQb, per_core_b = prep_edges(inputs["ei_pb"][0], inputs["ei_pb"][1], cfg)

    key = (tuple(sorted(cfg.items())), CQp, CQb, reps)
    if key not in _CACHE:
        _CACHE[key] = build_program(cfg, CQp, CQb, reps)
    nc = _CACHE[key]

    in_maps = build_in_maps(inputs, cfg, CQp, per_core_p, CQb, per_core_b)

    res = run_bass_kernel_spmd(nc, in_maps, list(range(n_cores)), trace=trace)
    outs = [res.results[c]["out"] for c in range(n_cores)]  # [2, shard] each
    full = np.concatenate(outs, axis=1)[:, :N].T.astype(np.float32)
    return np.ascontiguousarray(full), res


def kernel(**inputs):
    out, _ = _run(inputs, full_cfg(), trace=False)
    return out
